# revision 46
# baseline (speedup 1.0000x reference)
"""Causal self-attention (dense transformer block) on 8 trn2 NeuronCores.

Sharding: tensor-parallel over heads. Each core owns 2 of the 16 heads:
  - qkv projection: column-slice of W_qkv (128 cols per core)
  - attention for its (2 heads x 2 batches) = 4 (b,h) pairs
  - out projection: row-slice of W_out -> partial y [4096, 1024] (f32)
Host sums the 8 partial y's and adds b_out (+ the v-bias term folded
through W_out, since softmax rows sum to 1).

Device pipeline (bf16 matmuls, fp32 accumulation). The emission order is
a hand-interleaved schedule: projection and out-projection work is split
into self-contained ~0.5-1.7us "fill units" that are emitted between the
k-tile iterations of the attention loop, so the PE instruction stream
always has independent work queued while the per-tile exp runs on the
(slower) Activation engine.

  proj unit:   qT/kT[hd, s] = W.T @ xT (8 accum MMs + bias via DVE
               eviction); v unit likewise -> vstage; trans unit:
               PE-transpose of vstage -> v_aug [v_h0 | 1 | v_h1 | 1]
               (one gap-skipping DVE copy per k-tile)
  attn block:  per k-tile (diagonal tiles first): sT[k, q] both heads in
               one PSUM tile; the causal mask is applied ON THE PE by
               accumulating -65536 onto the strictly-upper triangle of
               diagonal tiles BEFORE the score matmul (exp underflows
               those entries to exactly 0 -- no gpsimd op on the
               exp->PV critical path); one exp per k-tile (ScalarE, no
               max subtraction: scores are bounded); PV: attnT[65, q] +=
               v_aug.T @ PT (row 64 = softmax denominators), deferred
               three k-tiles so exp latency never stalls the PE.
  norm:        immediately after the k-loop: one batched exact
               reciprocal from PSUM (reciprocal_approx_fast reads
               garbage from PSUM on HW) + per-head gpsimd
               partition_broadcast + DVE mul -> at_bj (bf16, SBUF).
               The last block's norm+out-proj is pipelined per q-tile.
  out-proj:    emitted as age-gated fill units: y[q, e] = at_bj.T @
               W_out rows, evicted to bf16 SBUF (DVE/ScalarE split),
               DMA'd per 128-row tile.
"""

import sys

if "/opt/trn_rl_repo" not in sys.path:
    sys.path.insert(0, "/opt/trn_rl_repo")

from collections import deque

import numpy as np
import ml_dtypes

import concourse.bass as bass
import concourse.tile as tile
from concourse import bacc, mybir
from concourse.bass_utils import run_bass_kernel_spmd
from concourse.masks import make_identity

BF16 = mybir.dt.bfloat16
F32 = mybir.dt.float32
AF = mybir.ActivationFunctionType

N_EMBED = 1024
N_HEAD = 16
HEAD_DIM = 64
N_CORES = 8
HEADS_PER_CORE = N_HEAD // N_CORES          # 2
DCORE = HEADS_PER_CORE * HEAD_DIM           # 128 head-dims per core
B = 2
S = 2048                                    # seq len per batch
QB = 512                                    # q-block (moving free dim)
KT = 128                                    # k-tile (contraction tile)
DT = N_EMBED // 128                         # 8 d-tiles for projections
SCALE = 1.0 / 8.0                           # 1/sqrt(HEAD_DIM)
VW = HEAD_DIM + 1                           # v_aug slice width per head
H = HEADS_PER_CORE


def build_program(seq=S):
    """Build the per-core Bass program (identical on all cores; SPMD)."""
    s_tot = B * seq                 # total rows across batches
    n_qb = seq // QB                # q-blocks per batch
    n_kt = seq // KT                # k-tiles per batch
    kt_per_qb = QB // KT            # 4

    nc = bacc.Bacc("TRN2", target_bir_lowering=False, debug=False,
                   num_devices=N_CORES)

    xT = nc.dram_tensor("xT", [N_EMBED, s_tot], BF16, kind="ExternalInput")
    wq = nc.dram_tensor("wq", [N_EMBED, DCORE], BF16, kind="ExternalInput")
    wk = nc.dram_tensor("wk", [N_EMBED, DCORE], BF16, kind="ExternalInput")
    wv = nc.dram_tensor("wv", [N_EMBED, DCORE], BF16, kind="ExternalInput")
    bq = nc.dram_tensor("bq", [DCORE, 1], F32, kind="ExternalInput")
    bk = nc.dram_tensor("bk", [DCORE, 1], F32, kind="ExternalInput")
    wout = nc.dram_tensor("wout", [DCORE, N_EMBED], BF16, kind="ExternalInput")
    y = nc.dram_tensor("y", [s_tot, N_EMBED], BF16, kind="ExternalOutput")

    xT_r = xT.ap().rearrange("(t p) s -> p t s", p=128)

    with (
        tile.TileContext(nc) as tc,
        tc.tile_pool(name="singles", bufs=1) as singles,
        # PSUM (8 banks): sy 2x[128,2,512]=4, attn 1x[65,2,512]=2, aux 2x1=2
        tc.tile_pool(name="sy_ps", bufs=2, space="PSUM") as sy_pool,
        tc.tile_pool(name="attn_ps", bufs=1, space="PSUM") as attn_pool,
        tc.tile_pool(name="aux_ps", bufs=2, space="PSUM") as aux_pool,
        tc.tile_pool(name="vstage", bufs=3) as vstage_pool,
        tc.tile_pool(name="pt_sb", bufs=8) as pt_pool,
        tc.tile_pool(name="rec_sb", bufs=4) as rec_pool,
        tc.tile_pool(name="bc_sb", bufs=4) as bc_pool,
        tc.tile_pool(name="at_sb", bufs=4) as at_pool,
        tc.tile_pool(name="ysb_sb", bufs=10) as ysb_pool,
    ):
        # ---- persistent SBUF tensors ----
        xT_sb = singles.tile([128, DT, s_tot], BF16)
        wq_sb = singles.tile([128, DT, DCORE], BF16)
        wk_sb = singles.tile([128, DT, DCORE], BF16)
        wv_sb = singles.tile([128, DT, DCORE], BF16)
        bq_sb = singles.tile([DCORE, 1], F32)
        bk_sb = singles.tile([DCORE, 1], F32)
        wout_sb = singles.tile([DCORE, N_EMBED], BF16)
        qT_sb = singles.tile([DCORE, s_tot], BF16)
        kT_sb = singles.tile([DCORE, s_tot], BF16)
        # v_aug per global k-tile: [v_h0(0:64) | 1(64) | v_h1(65:129) | 1(129)]
        v_aug = singles.tile([128, B * n_kt, 2 * VW], BF16)
        ident_sb = singles.tile([128, 128], BF16)
        negI_sb = singles.tile([128, 128], BF16)
        tri_sb = singles.tile([128, 128], BF16)

        # ---- input DMAs ----
        # x blocks issue on the SP sequencer (first DMA out the door is
        # x block 0); weights issue in parallel on the Activation and DVE
        # sequencers so the projections can start ~4us in.
        # block 0 split in quarters so the first projection starts earlier
        for qd in range(4):
            nc.sync.dma_start(out=xT_sb[:, 2 * qd:2 * qd + 2, 0:QB],
                              in_=xT_r[:, 2 * qd:2 * qd + 2, 0:QB])
        for hh in range(2):
            nc.sync.dma_start(out=xT_sb[:, 4 * hh:4 * hh + 4, QB:2 * QB],
                              in_=xT_r[:, 4 * hh:4 * hh + 4, QB:2 * QB])
        for sb in range(2, s_tot // QB):
            nc.sync.dma_start(out=xT_sb[:, :, sb * QB:(sb + 1) * QB],
                              in_=xT_r[:, :, sb * QB:(sb + 1) * QB])
        nc.scalar.dma_start(out=wq_sb,
                            in_=wq.ap().rearrange("(t p) h -> p t h", p=128))
        nc.scalar.dma_start(out=wk_sb,
                            in_=wk.ap().rearrange("(t p) h -> p t h", p=128))
        nc.scalar.dma_start(out=wv_sb,
                            in_=wv.ap().rearrange("(t p) h -> p t h", p=128))
        nc.scalar.dma_start(out=bq_sb, in_=bq.ap())
        nc.scalar.dma_start(out=bk_sb, in_=bk.ap())
        nc.scalar.dma_start(out=wout_sb, in_=wout.ap())

        make_identity(nc, ident_sb)
        nc.vector.memset(v_aug[:, :, HEAD_DIM], 1.0)
        nc.vector.memset(v_aug[:, :, VW + HEAD_DIM], 1.0)
        # causal-mask helpers: negI = -65536*I; tri[r, c] = 1 iff r > c.
        # The mask matmul negI.T @ tri accumulates -65536 onto the
        # strictly-upper-triangular score entries of a diagonal k-tile, so
        # exp(scale*(s - 65536)) underflows to exactly 0 -- no gpsimd
        # affine_select on the exp->PV critical path.
        nc.vector.tensor_scalar_mul(negI_sb, ident_sb, -65536.0)
        nc.vector.memset(tri_sb, 1.0)
        nc.gpsimd.affine_select(
            out=tri_sb, in_=tri_sb,
            compare_op=mybir.AluOpType.is_ge, fill=0.0,
            base=-1, channel_multiplier=1, pattern=[[-1, 128]])

        # ------------------------------------------------------------------
        # fill units: self-contained closures (open+close their PSUM use)
        # ------------------------------------------------------------------

        def proj_units(sb):
            """Projection for 512-row block sb.

            Each qkv projection is split into two half-units of 4 accum
            MMs (~850ns PE each). The PSUM accumulation group stays open
            across the pair, which is safe because the units are adjacent
            in the fill queue: only attention work (sy/attn/pt pools) is
            emitted between them, never another aux-pool user.
            Returns [(pe_cost_ns, emit_fn), ...]."""
            sl = slice(sb * QB, (sb + 1) * QB)

            units = []
            boxes = {}
            for key, w_sb, b_sb, dst in (("q", wq_sb, bq_sb, qT_sb),
                                         ("k", wk_sb, bk_sb, kT_sb),
                                         ("v", wv_sb, None, None)):
                boxes[key] = {}

                def ua(key=key, w_sb=w_sb):
                    ps = aux_pool.tile([128, QB], F32, tag="aux", name="ps")
                    boxes[key]["ps"] = ps
                    for t in range(DT // 2):
                        nc.tensor.matmul(ps, lhsT=w_sb[:, t, :],
                                         rhs=xT_sb[:, t, sl],
                                         start=(t == 0), stop=False)

                def ub(key=key, w_sb=w_sb, b_sb=b_sb, dst=dst):
                    ps = boxes[key]["ps"]
                    for t in range(DT // 2, DT):
                        nc.tensor.matmul(ps, lhsT=w_sb[:, t, :],
                                         rhs=xT_sb[:, t, sl],
                                         start=False, stop=(t == DT - 1))
                    if dst is not None:
                        nc.vector.tensor_scalar_add(dst[:, sl], ps, b_sb)
                    else:
                        vstage = vstage_pool.tile([128, QB], BF16)
                        boxes["v"]["vstage"] = vstage
                        nc.vector.tensor_copy(vstage, ps)

                units += [(860, ua), (860, ub)]

            def u_trans():
                vstage = boxes["v"]["vstage"]
                for u in range(QB // 128):
                    kt_gl = (QB // 128) * sb + u
                    tr = aux_pool.tile([128, 128], BF16, tag="aux", name="tr")
                    nc.tensor.transpose(
                        tr, vstage[:, u * 128:(u + 1) * 128], ident_sb)
                    # one copy; dst AP skips the two "ones" columns (64, 129)
                    dst = v_aug[:, kt_gl, :].rearrange(
                        "p (g w) -> p g w", g=2)[:, :, 0:HEAD_DIM]
                    src = tr[:, :].rearrange("p (g w) -> p g w", g=2)
                    nc.vector.tensor_copy(dst, src)

            units.append((220, u_trans))
            # kinds: q-eviction closes at unit 1, block fully done at 6
            return units

        def outproj_units(b_i, j, at_bj):
            """Out-projection for q-block j of batch b_i: 4 fill units.

            Evictions go to DVE only, keeping the Activation engine
            exp-exclusive."""
            units = []
            for qt in range(QB // 128):
                def u(qt=qt, at_bj=at_bj):
                    at = at_bj[:, qt * 128:(qt + 1) * 128]
                    row0 = b_i * seq + j * QB + qt * 128
                    ysb = ysb_pool.tile([128, N_EMBED], BF16, tag="ysb",
                                        name="ysb")
                    for uu in range(N_EMBED // QB):
                        yp = aux_pool.tile([128, QB], F32, tag="aux",
                                           name="yp")
                        nc.tensor.matmul(yp, lhsT=at,
                                         rhs=wout_sb[:, uu * QB:(uu + 1) * QB],
                                         start=True, stop=True)
                        if uu == 0:
                            nc.vector.tensor_copy(ysb[:, 0:QB], yp)
                        else:
                            nc.scalar.copy(ysb[:, QB:2 * QB], yp)
                    nc.sync.dma_start(out=y.ap()[row0:row0 + 128, :], in_=ysb)
                units.append((440, u))
            return units

        # ------------------------------------------------------------------
        # schedule state: proj units (tagged by block) + outproj fills.
        # Fills are rationed by a PE-deficit debt counter so they last
        # through the late (fill-starved) attention blocks.
        # ------------------------------------------------------------------
        proj_q = deque()
        for sbk in range(B * n_qb):
            us = proj_units(sbk)
            for i, (cost, u) in enumerate(us):
                proj_q.append((sbk if i == len(us) - 1 else None, cost, u))
        fills = deque()             # (ready_tick, cost, fn)
        proj_done = -1
        debt = 0.0                  # ns of PE idle to cover with fills
        tick = 0                    # global k-tile iteration counter

        def pull_one():
            nonlocal proj_done, debt
            if proj_q:
                tag, cost, u = proj_q.popleft()
                u()
                if tag is not None:
                    proj_done = tag
                debt -= cost
                return True
            if fills and fills[0][0] <= tick:
                _, cost, u = fills.popleft()
                u()
                debt -= cost
                return True
            return False

        def add_debt(ns):
            nonlocal debt
            debt += ns
            while debt > 0 and pull_one():
                pass

        def drain_proj(k):
            while proj_done < k and proj_q:
                pull_one()

        def attn_block(b_i, j):
            """Scores/exp/mask/PV for q-block j of batch b_i, with fills.

            Diagonal k-tiles first (baseline order) so the final PV is a
            full-range stop for j>0."""
            qsl = slice(b_i * seq + j * QB, b_i * seq + (j + 1) * QB)
            attn65 = attn_pool.tile([VW, H, QB], F32, tag="attn",
                                    name="attn65")
            kts = list(range(kt_per_qb * j, kt_per_qb * (j + 1))) + \
                list(range(0, kt_per_qb * j))

            def emit_pv(kt, pt, off, pos):
                for h in range(H):
                    nc.tensor.matmul(
                        attn65[:, h, off:],
                        lhsT=v_aug[:, b_i * n_kt + kt, VW * h:VW * (h + 1)],
                        rhs=pt[:, h, off:],
                        start=(pos == 0), stop=(pos == len(kts) - 1))

            pend = deque()
            for pos, kt in enumerate(kts):
                ks = slice(b_i * seq + kt * 128, b_i * seq + kt * 128 + 128)
                d = kt - kt_per_qb * j
                off = 128 * d if d >= 0 else 0   # first valid q column
                pt = pt_pool.tile([128, H, QB], BF16, tag="pt", name="pt")
                s_ps = sy_pool.tile([128, H, QB], F32, tag="sy", name="s_ps")
                for h in range(H):
                    hsl = slice(HEAD_DIM * h, HEAD_DIM * (h + 1))
                    if d >= 0:
                        # diagonal: write -65536 onto the masked triangle
                        # FIRST (start=True), scores then accumulate onto
                        # it ([off:off+128)) / overwrite (has_written=0
                        # beyond). exp underflows masked entries to 0.
                        nc.tensor.matmul(
                            s_ps[:, h, off:off + 128],
                            lhsT=negI_sb, rhs=tri_sb,
                            start=True, stop=False, skip_group_check=True)
                    nc.tensor.matmul(
                        s_ps[:, h, off:],
                        lhsT=kT_sb[hsl, ks],
                        rhs=qT_sb[hsl, qsl.start + off:qsl.stop],
                        start=(d < 0), stop=True, skip_group_check=True)
                nc.scalar.activation(pt[:, :, off:], s_ps[:, :, off:],
                                     AF.Exp, scale=SCALE)
                # PV deferred three k-tiles: hides exp (ScalarE), the
                # diagonal mask (gpsimd), and the previous block's norm
                # chain from the PE stream
                pend.append((kt, pt, off, pos))
                if len(pend) > 2:
                    emit_pv(*pend.popleft())
                # PE deficit this iter: exp time minus attn PE time
                nonlocal tick
                tick += 1
                cols = QB - off
                pcols = QB - pend[0][2] if pend else 0
                exp_ns = 1.667 * cols + 190
                pe_ns = 0.857 * (cols + pcols)
                add_debt(exp_ns - pe_ns)
            while pend:
                emit_pv(*pend.popleft())
            return attn65

        def norm_block(attn65):
            """Batched reciprocal + per-head broadcast/mul -> at_bj bf16."""
            rf = rec_pool.tile([1, H, QB], F32, tag="rf", name="rf")
            # NOTE: reciprocal_approx_fast reads garbage from PSUM on HW;
            # the exact reciprocal handles PSUM sources correctly.
            nc.vector.reciprocal(rf, attn65[HEAD_DIM:VW, :, :])
            at_bj = at_pool.tile([DCORE, QB], BF16, tag="at", name="at_bj")
            for h in range(H):
                bc_sb = bc_pool.tile([HEAD_DIM, QB], F32, tag=f"bc{h}",
                                     name=f"bc{h}")
                nc.gpsimd.partition_broadcast(bc_sb, rf[0:1, h, :])
                nc.vector.tensor_mul(
                    at_bj[HEAD_DIM * h:HEAD_DIM * (h + 1), :],
                    attn65[0:HEAD_DIM, h, :], bc_sb)
            return at_bj

        # ------------------------------------------------------------------
        # main schedule
        # ------------------------------------------------------------------
        def norm_outproj_pipelined(b_i, j, attn65):
            """Last block: per-q-tile norm + immediate out-proj so the
            tail chain rf->bcast->mul->MM->evict->DMA pipelines across
            q-tiles instead of serializing once at the end."""
            at_bj = at_pool.tile([DCORE, QB], BF16, tag="at", name="at_bj")
            for qt in range(QB // 128):
                cs = slice(qt * 128, (qt + 1) * 128)
                rf = rec_pool.tile([1, H, 128], F32, tag="rfq", name="rfq")
                nc.vector.reciprocal(rf, attn65[HEAD_DIM:VW, :, cs])
                for h in range(H):
                    bc_sb = bc_pool.tile([HEAD_DIM, 128], F32,
                                         tag=f"bcq{h}", name=f"bcq{h}")
                    nc.gpsimd.partition_broadcast(bc_sb, rf[0:1, h, :])
                    nc.vector.tensor_mul(
                        at_bj[HEAD_DIM * h:HEAD_DIM * (h + 1), cs],
                        attn65[0:HEAD_DIM, h, cs], bc_sb)
                row0 = b_i * seq + j * QB + qt * 128
                ysb = ysb_pool.tile([128, N_EMBED], BF16, tag="ysb",
                                    name="ysb")
                for uu in range(N_EMBED // QB):
                    yp = aux_pool.tile([128, QB], F32, tag="aux", name="yp")
                    nc.tensor.matmul(yp, lhsT=at_bj[:, cs],
                                     rhs=wout_sb[:, uu * QB:(uu + 1) * QB],
                                     start=True, stop=True)
                    if uu == 0:
                        nc.vector.tensor_copy(ysb[:, 0:QB], yp)
                    else:
                        nc.scalar.copy(ysb[:, QB:2 * QB], yp)
                nc.sync.dma_start(out=y.ap()[row0:row0 + 128, :], in_=ysb)

        for b_i in range(B):
            for j in range(n_qb):
                drain_proj(4 * b_i + j)
                attn65 = attn_block(b_i, j)
                if (b_i, j) == (B - 1, n_qb - 1):
                    while proj_q or fills:
                        pull_one()
                    norm_outproj_pipelined(b_i, j, attn65)
                else:
                    at_bj = norm_block(attn65)
                    # age-gate: give the rf->bcast->mul chain ~4 k-tile
                    # iterations before out-proj consumes at_bj
                    for cost, u in outproj_units(b_i, j, at_bj):
                        fills.append((tick + 4, cost, u))
        while proj_q or fills:
            if not pull_one():
                tick += 1

    nc.compile()
    return nc


_CACHE = {}


def _get_program(seq=S):
    if seq not in _CACHE:
        _CACHE[seq] = build_program(seq)
    return _CACHE[seq]


def make_in_maps(x, W_qkv, b_qkv, seq=S):
    bf16 = ml_dtypes.bfloat16
    s_tot = B * seq
    xT = np.ascontiguousarray(
        x.reshape(s_tot, N_EMBED).T).astype(bf16)
    in_maps = []
    for c in range(N_CORES):
        csl = slice(DCORE * c, DCORE * (c + 1))
        in_maps.append({
            "xT": xT,
            "wq": np.ascontiguousarray(W_qkv[:, csl]).astype(bf16),
            "wk": np.ascontiguousarray(W_qkv[:, N_EMBED:][:, csl]).astype(bf16),
            "wv": np.ascontiguousarray(W_qkv[:, 2 * N_EMBED:][:, csl]).astype(bf16),
            "bq": np.ascontiguousarray(
                b_qkv[csl].reshape(DCORE, 1)).astype(np.float32),
            "bk": np.ascontiguousarray(
                b_qkv[N_EMBED:][csl].reshape(DCORE, 1)).astype(np.float32),
            "wout": None,  # filled by caller
        })
    return in_maps


def kernel(x, W_qkv, b_qkv, W_out, b_out):
    x = np.asarray(x, dtype=np.float32)
    W_qkv = np.asarray(W_qkv, dtype=np.float32)
    b_qkv = np.asarray(b_qkv, dtype=np.float32)
    W_out = np.asarray(W_out, dtype=np.float32)
    b_out = np.asarray(b_out, dtype=np.float32)

    nc = _get_program(S)
    in_maps = make_in_maps(x, W_qkv, b_qkv, S)
    bf16 = ml_dtypes.bfloat16
    for c in range(N_CORES):
        csl = slice(DCORE * c, DCORE * (c + 1))
        in_maps[c]["wout"] = np.ascontiguousarray(W_out[csl, :]).astype(bf16)

    res = run_bass_kernel_spmd(nc, in_maps, core_ids=list(range(N_CORES)))
    y = np.zeros((B * S, N_EMBED), dtype=np.float32)
    for r in res.results:
        y += r["y"].astype(np.float32)
    # bias + v-bias folded through W_out (softmax rows sum to 1)
    y += b_out[None, :] + b_qkv[2 * N_EMBED:] @ W_out
    return y.reshape(B, S, N_EMBED)


# revision 47
# speedup vs baseline: 1.0015x; 1.0015x over previous
"""Causal self-attention (dense transformer block) on 8 trn2 NeuronCores.

Sharding: tensor-parallel over heads. Each core owns 2 of the 16 heads:
  - qkv projection: column-slice of W_qkv (128 cols per core)
  - attention for its (2 heads x 2 batches) = 4 (b,h) pairs
  - out projection: row-slice of W_out -> partial y [4096, 1024] (f32)
Host sums the 8 partial y's and adds b_out (+ the v-bias term folded
through W_out, since softmax rows sum to 1).

Device pipeline (bf16 matmuls, fp32 accumulation). The emission order is
a hand-interleaved schedule: projection and out-projection work is split
into self-contained ~0.5-1.7us "fill units" that are emitted between the
k-tile iterations of the attention loop, so the PE instruction stream
always has independent work queued while the per-tile exp runs on the
(slower) Activation engine.

  proj unit:   qT/kT[hd, s] = W.T @ xT (8 accum MMs + bias via DVE
               eviction); v unit likewise -> vstage; trans unit:
               PE-transpose of vstage -> v_aug [v_h0 | 1 | v_h1 | 1]
               (one gap-skipping DVE copy per k-tile)
  attn block:  per k-tile (diagonal tiles first): sT[k, q] both heads in
               one PSUM tile; the causal mask is applied ON THE PE by
               accumulating -65536 onto the strictly-upper triangle of
               diagonal tiles BEFORE the score matmul (exp underflows
               those entries to exactly 0 -- no gpsimd op on the
               exp->PV critical path); one exp per k-tile (ScalarE, no
               max subtraction: scores are bounded); PV: attnT[65, q] +=
               v_aug.T @ PT (row 64 = softmax denominators), deferred
               three k-tiles so exp latency never stalls the PE.
  norm:        immediately after the k-loop: one batched exact
               reciprocal from PSUM (reciprocal_approx_fast reads
               garbage from PSUM on HW) + per-head gpsimd
               partition_broadcast + DVE mul -> at_bj (bf16, SBUF).
               The last block's norm+out-proj is pipelined per q-tile.
  out-proj:    emitted as age-gated fill units: y[q, e] = at_bj.T @
               W_out rows, evicted to bf16 SBUF (DVE/ScalarE split),
               DMA'd per 128-row tile.
"""

import sys

if "/opt/trn_rl_repo" not in sys.path:
    sys.path.insert(0, "/opt/trn_rl_repo")

from collections import deque

import numpy as np
import ml_dtypes

import concourse.bass as bass
import concourse.tile as tile
from concourse import bacc, mybir
from concourse.bass_utils import run_bass_kernel_spmd
from concourse.masks import make_identity

BF16 = mybir.dt.bfloat16
F32 = mybir.dt.float32
AF = mybir.ActivationFunctionType

N_EMBED = 1024
N_HEAD = 16
HEAD_DIM = 64
N_CORES = 8
HEADS_PER_CORE = N_HEAD // N_CORES          # 2
DCORE = HEADS_PER_CORE * HEAD_DIM           # 128 head-dims per core
B = 2
S = 2048                                    # seq len per batch
QB = 512                                    # q-block (moving free dim)
KT = 128                                    # k-tile (contraction tile)
DT = N_EMBED // 128                         # 8 d-tiles for projections
SCALE = 1.0 / 8.0                           # 1/sqrt(HEAD_DIM)
VW = HEAD_DIM + 1                           # v_aug slice width per head
H = HEADS_PER_CORE


def build_program(seq=S):
    """Build the per-core Bass program (identical on all cores; SPMD)."""
    s_tot = B * seq                 # total rows across batches
    n_qb = seq // QB                # q-blocks per batch
    n_kt = seq // KT                # k-tiles per batch
    kt_per_qb = QB // KT            # 4

    nc = bacc.Bacc("TRN2", target_bir_lowering=False, debug=False,
                   num_devices=N_CORES)

    xT = nc.dram_tensor("xT", [N_EMBED, s_tot], BF16, kind="ExternalInput")
    wq = nc.dram_tensor("wq", [N_EMBED, DCORE], BF16, kind="ExternalInput")
    wk = nc.dram_tensor("wk", [N_EMBED, DCORE], BF16, kind="ExternalInput")
    wv = nc.dram_tensor("wv", [N_EMBED, DCORE], BF16, kind="ExternalInput")
    bq = nc.dram_tensor("bq", [DCORE, 1], F32, kind="ExternalInput")
    bk = nc.dram_tensor("bk", [DCORE, 1], F32, kind="ExternalInput")
    wout = nc.dram_tensor("wout", [DCORE, N_EMBED], BF16, kind="ExternalInput")
    y = nc.dram_tensor("y", [s_tot, N_EMBED], BF16, kind="ExternalOutput")

    xT_r = xT.ap().rearrange("(t p) s -> p t s", p=128)

    with (
        tile.TileContext(nc) as tc,
        tc.tile_pool(name="singles", bufs=1) as singles,
        # PSUM (8 banks): sy 2x[128,2,512]=4, attn 1x[65,2,512]=2, aux 2x1=2
        tc.tile_pool(name="sy_ps", bufs=2, space="PSUM") as sy_pool,
        tc.tile_pool(name="attn_ps", bufs=1, space="PSUM") as attn_pool,
        tc.tile_pool(name="aux_ps", bufs=2, space="PSUM") as aux_pool,
        tc.tile_pool(name="vstage", bufs=3) as vstage_pool,
        tc.tile_pool(name="pt_sb", bufs=8) as pt_pool,
        tc.tile_pool(name="rec_sb", bufs=4) as rec_pool,
        tc.tile_pool(name="bc_sb", bufs=4) as bc_pool,
        tc.tile_pool(name="at_sb", bufs=4) as at_pool,
        tc.tile_pool(name="ysb_sb", bufs=10) as ysb_pool,
    ):
        # ---- persistent SBUF tensors ----
        xT_sb = singles.tile([128, DT, s_tot], BF16)
        wq_sb = singles.tile([128, DT, DCORE], BF16)
        wk_sb = singles.tile([128, DT, DCORE], BF16)
        wv_sb = singles.tile([128, DT, DCORE], BF16)
        bq_sb = singles.tile([DCORE, 1], F32)
        bk_sb = singles.tile([DCORE, 1], F32)
        wout_sb = singles.tile([DCORE, N_EMBED], BF16)
        qT_sb = singles.tile([DCORE, s_tot], BF16)
        kT_sb = singles.tile([DCORE, s_tot], BF16)
        # v_aug per global k-tile: [v_h0(0:64) | 1(64) | v_h1(65:129) | 1(129)]
        v_aug = singles.tile([128, B * n_kt, 2 * VW], BF16)
        ident_sb = singles.tile([128, 128], BF16)
        negI_sb = singles.tile([128, 128], BF16)
        tri_sb = singles.tile([128, 128], BF16)

        # ---- input DMAs ----
        # x blocks issue on the SP sequencer (first DMA out the door is
        # x block 0); weights issue in parallel on the Activation and DVE
        # sequencers so the projections can start ~4us in.
        # block 0 split in quarters so the first projection starts earlier
        for qd in range(4):
            nc.sync.dma_start(out=xT_sb[:, 2 * qd:2 * qd + 2, 0:QB],
                              in_=xT_r[:, 2 * qd:2 * qd + 2, 0:QB])
        for hh in range(2):
            nc.sync.dma_start(out=xT_sb[:, 4 * hh:4 * hh + 4, QB:2 * QB],
                              in_=xT_r[:, 4 * hh:4 * hh + 4, QB:2 * QB])
        for sb in range(2, s_tot // QB):
            nc.sync.dma_start(out=xT_sb[:, :, sb * QB:(sb + 1) * QB],
                              in_=xT_r[:, :, sb * QB:(sb + 1) * QB])
        nc.scalar.dma_start(out=wq_sb,
                            in_=wq.ap().rearrange("(t p) h -> p t h", p=128))
        nc.scalar.dma_start(out=wk_sb,
                            in_=wk.ap().rearrange("(t p) h -> p t h", p=128))
        nc.scalar.dma_start(out=wv_sb,
                            in_=wv.ap().rearrange("(t p) h -> p t h", p=128))
        nc.scalar.dma_start(out=bq_sb, in_=bq.ap())
        nc.scalar.dma_start(out=bk_sb, in_=bk.ap())
        nc.scalar.dma_start(out=wout_sb, in_=wout.ap())

        make_identity(nc, ident_sb)
        nc.vector.memset(v_aug[:, :, HEAD_DIM], 1.0)
        nc.vector.memset(v_aug[:, :, VW + HEAD_DIM], 1.0)
        # causal-mask helpers: negI = -65536*I; tri[r, c] = 1 iff r > c.
        # The mask matmul negI.T @ tri accumulates -65536 onto the
        # strictly-upper-triangular score entries of a diagonal k-tile, so
        # exp(scale*(s - 65536)) underflows to exactly 0 -- no gpsimd
        # affine_select on the exp->PV critical path.
        nc.vector.tensor_scalar_mul(negI_sb, ident_sb, -65536.0)
        nc.vector.memset(tri_sb, 1.0)
        nc.gpsimd.affine_select(
            out=tri_sb, in_=tri_sb,
            compare_op=mybir.AluOpType.is_ge, fill=0.0,
            base=-1, channel_multiplier=1, pattern=[[-1, 128]])

        # ------------------------------------------------------------------
        # fill units: self-contained closures (open+close their PSUM use)
        # ------------------------------------------------------------------

        def proj_units(sb):
            """Projection for 512-row block sb.

            Each qkv projection is split into two half-units of 4 accum
            MMs (~850ns PE each). The PSUM accumulation group stays open
            across the pair, which is safe because the units are adjacent
            in the fill queue: only attention work (sy/attn/pt pools) is
            emitted between them, never another aux-pool user.
            Returns [(pe_cost_ns, emit_fn), ...]."""
            sl = slice(sb * QB, (sb + 1) * QB)

            units = []
            boxes = {}
            for key, w_sb, b_sb, dst in (("q", wq_sb, bq_sb, qT_sb),
                                         ("k", wk_sb, bk_sb, kT_sb),
                                         ("v", wv_sb, None, None)):
                boxes[key] = {}

                def ua(key=key, w_sb=w_sb):
                    ps = aux_pool.tile([128, QB], F32, tag="aux", name="ps")
                    boxes[key]["ps"] = ps
                    for t in range(DT // 2):
                        nc.tensor.matmul(ps, lhsT=w_sb[:, t, :],
                                         rhs=xT_sb[:, t, sl],
                                         start=(t == 0), stop=False)

                def ub(key=key, w_sb=w_sb, b_sb=b_sb, dst=dst):
                    ps = boxes[key]["ps"]
                    for t in range(DT // 2, DT):
                        nc.tensor.matmul(ps, lhsT=w_sb[:, t, :],
                                         rhs=xT_sb[:, t, sl],
                                         start=False, stop=(t == DT - 1))
                    if dst is not None:
                        nc.vector.tensor_scalar_add(dst[:, sl], ps, b_sb)
                    else:
                        vstage = vstage_pool.tile([128, QB], BF16)
                        boxes["v"]["vstage"] = vstage
                        nc.vector.tensor_copy(vstage, ps)

                units += [(860, ua), (860, ub)]

            def u_trans():
                vstage = boxes["v"]["vstage"]
                for u in range(QB // 128):
                    kt_gl = (QB // 128) * sb + u
                    tr = aux_pool.tile([128, 128], BF16, tag="aux", name="tr")
                    nc.tensor.transpose(
                        tr, vstage[:, u * 128:(u + 1) * 128], ident_sb)
                    # one copy; dst AP skips the two "ones" columns (64, 129)
                    dst = v_aug[:, kt_gl, :].rearrange(
                        "p (g w) -> p g w", g=2)[:, :, 0:HEAD_DIM]
                    src = tr[:, :].rearrange("p (g w) -> p g w", g=2)
                    nc.vector.tensor_copy(dst, src)

            units.append((220, u_trans))
            # kinds: q-eviction closes at unit 1, block fully done at 6
            return units

        def outproj_units(b_i, j, at_bj):
            """Out-projection for q-block j of batch b_i: 4 fill units.

            Evictions go to DVE only, keeping the Activation engine
            exp-exclusive."""
            units = []
            for qt in range(QB // 128):
                def u(qt=qt, at_bj=at_bj):
                    at = at_bj[:, qt * 128:(qt + 1) * 128]
                    row0 = b_i * seq + j * QB + qt * 128
                    ysb = ysb_pool.tile([128, N_EMBED], BF16, tag="ysb",
                                        name="ysb")
                    for uu in range(N_EMBED // QB):
                        yp = aux_pool.tile([128, QB], F32, tag="aux",
                                           name="yp")
                        nc.tensor.matmul(yp, lhsT=at,
                                         rhs=wout_sb[:, uu * QB:(uu + 1) * QB],
                                         start=True, stop=True)
                        if uu == 0:
                            nc.vector.tensor_copy(ysb[:, 0:QB], yp)
                        else:
                            nc.scalar.copy(ysb[:, QB:2 * QB], yp)
                    nc.sync.dma_start(out=y.ap()[row0:row0 + 128, :], in_=ysb)
                units.append((440, u))
            return units

        # ------------------------------------------------------------------
        # schedule state: proj units (tagged by block) + outproj fills.
        # Fills are rationed by a PE-deficit debt counter so they last
        # through the late (fill-starved) attention blocks.
        # ------------------------------------------------------------------
        proj_q = deque()
        for sbk in range(B * n_qb):
            us = proj_units(sbk)
            for i, (cost, u) in enumerate(us):
                proj_q.append((sbk if i == len(us) - 1 else None, cost, u))
        fills = deque()             # (ready_tick, cost, fn)
        proj_done = -1
        debt = 0.0                  # ns of PE idle to cover with fills
        tick = 0                    # global k-tile iteration counter

        def pull_one():
            nonlocal proj_done, debt
            if proj_q:
                tag, cost, u = proj_q.popleft()
                u()
                if tag is not None:
                    proj_done = tag
                debt -= cost
                return True
            if fills and fills[0][0] <= tick:
                _, cost, u = fills.popleft()
                u()
                debt -= cost
                return True
            return False

        def add_debt(ns):
            nonlocal debt
            debt += ns
            while debt > 0 and pull_one():
                pass

        def drain_proj(k):
            while proj_done < k and proj_q:
                pull_one()

        def attn_block(b_i, j):
            """Scores/exp/mask/PV for q-block j of batch b_i, with fills.

            Diagonal k-tiles first (baseline order) so the final PV is a
            full-range stop for j>0."""
            qsl = slice(b_i * seq + j * QB, b_i * seq + (j + 1) * QB)
            attn65 = attn_pool.tile([VW, H, QB], F32, tag="attn",
                                    name="attn65")
            kts = list(range(kt_per_qb * j, kt_per_qb * (j + 1))) + \
                list(range(0, kt_per_qb * j))

            def emit_pv(kt, pt, off, pos):
                for h in range(H):
                    nc.tensor.matmul(
                        attn65[:, h, off:],
                        lhsT=v_aug[:, b_i * n_kt + kt, VW * h:VW * (h + 1)],
                        rhs=pt[:, h, off:],
                        start=(pos == 0), stop=(pos == len(kts) - 1))

            pend = deque()
            for pos, kt in enumerate(kts):
                ks = slice(b_i * seq + kt * 128, b_i * seq + kt * 128 + 128)
                d = kt - kt_per_qb * j
                off = 128 * d if d >= 0 else 0   # first valid q column
                pt = pt_pool.tile([128, H, QB], BF16, tag="pt", name="pt")
                s_ps = sy_pool.tile([128, H, QB], F32, tag="sy", name="s_ps")
                for h in range(H):
                    hsl = slice(HEAD_DIM * h, HEAD_DIM * (h + 1))
                    if d >= 0:
                        # diagonal: write -65536 onto the masked triangle
                        # FIRST (start=True), scores then accumulate onto
                        # it ([off:off+128)) / overwrite (has_written=0
                        # beyond). exp underflows masked entries to 0.
                        nc.tensor.matmul(
                            s_ps[:, h, off:off + 128],
                            lhsT=negI_sb, rhs=tri_sb,
                            start=True, stop=False, skip_group_check=True)
                    nc.tensor.matmul(
                        s_ps[:, h, off:],
                        lhsT=kT_sb[hsl, ks],
                        rhs=qT_sb[hsl, qsl.start + off:qsl.stop],
                        start=(d < 0), stop=True, skip_group_check=True)
                nc.scalar.activation(pt[:, :, off:], s_ps[:, :, off:],
                                     AF.Exp, scale=SCALE)
                # PV deferred three k-tiles: hides exp (ScalarE), the
                # diagonal mask (gpsimd), and the previous block's norm
                # chain from the PE stream
                pend.append((kt, pt, off, pos))
                if len(pend) > 2:
                    emit_pv(*pend.popleft())
                # PE deficit this iter: exp time minus attn PE time
                nonlocal tick
                tick += 1
                cols = QB - off
                pcols = QB - pend[0][2] if pend else 0
                exp_ns = 1.667 * cols + 190
                pe_ns = 0.857 * (cols + pcols)
                add_debt(exp_ns - pe_ns)
            while pend:
                emit_pv(*pend.popleft())
            return attn65

        def norm_block(attn65):
            """Batched reciprocal + per-head broadcast/mul -> at_bj bf16."""
            rf = rec_pool.tile([1, H, QB], F32, tag="rf", name="rf")
            # NOTE: reciprocal_approx_fast reads garbage from PSUM on HW;
            # the exact reciprocal handles PSUM sources correctly.
            nc.vector.reciprocal(rf, attn65[HEAD_DIM:VW, :, :])
            at_bj = at_pool.tile([DCORE, QB], BF16, tag="at", name="at_bj")
            for h in range(H):
                bc_sb = bc_pool.tile([HEAD_DIM, QB], F32, tag=f"bc{h}",
                                     name=f"bc{h}")
                nc.gpsimd.partition_broadcast(bc_sb, rf[0:1, h, :])
                nc.vector.tensor_mul(
                    at_bj[HEAD_DIM * h:HEAD_DIM * (h + 1), :],
                    attn65[0:HEAD_DIM, h, :], bc_sb)
            return at_bj

        # ------------------------------------------------------------------
        # main schedule
        # ------------------------------------------------------------------
        def norm_outproj_pipelined(b_i, j, attn65):
            """Last block: per-q-tile norm + immediate out-proj so the
            tail chain rf->bcast->mul->MM->evict->DMA pipelines across
            q-tiles instead of serializing once at the end."""
            at_bj = at_pool.tile([DCORE, QB], BF16, tag="at", name="at_bj")
            # hoist all reciprocals + broadcasts so DVE/Pool pipeline
            # them instead of each qt's rf queueing behind the previous
            # qt's muls/evictions
            bcs = {}
            for qt in range(QB // 128):
                cs = slice(qt * 128, (qt + 1) * 128)
                rf = rec_pool.tile([1, H, 128], F32, tag=f"rfq{qt % 2}",
                                   name="rfq")
                nc.vector.reciprocal(rf, attn65[HEAD_DIM:VW, :, cs])
                for h in range(H):
                    bc_sb = bc_pool.tile([HEAD_DIM, 128], F32,
                                         tag=f"bcq{qt % 2}{h}",
                                         name=f"bcq{h}")
                    nc.gpsimd.partition_broadcast(bc_sb, rf[0:1, h, :])
                    bcs[(qt, h)] = bc_sb
            for qt in range(QB // 128):
                cs = slice(qt * 128, (qt + 1) * 128)
                for h in range(H):
                    nc.vector.tensor_mul(
                        at_bj[HEAD_DIM * h:HEAD_DIM * (h + 1), cs],
                        attn65[0:HEAD_DIM, h, cs], bcs[(qt, h)])
                row0 = b_i * seq + j * QB + qt * 128
                ysb = ysb_pool.tile([128, N_EMBED], BF16, tag="ysb",
                                    name="ysb")
                for uu in range(N_EMBED // QB):
                    yp = aux_pool.tile([128, QB], F32, tag="aux", name="yp")
                    nc.tensor.matmul(yp, lhsT=at_bj[:, cs],
                                     rhs=wout_sb[:, uu * QB:(uu + 1) * QB],
                                     start=True, stop=True)
                    if uu == 0:
                        nc.vector.tensor_copy(ysb[:, 0:QB], yp)
                    else:
                        nc.scalar.copy(ysb[:, QB:2 * QB], yp)
                nc.sync.dma_start(out=y.ap()[row0:row0 + 128, :], in_=ysb)

        for b_i in range(B):
            for j in range(n_qb):
                drain_proj(4 * b_i + j)
                attn65 = attn_block(b_i, j)
                if (b_i, j) == (B - 1, n_qb - 1):
                    while proj_q or fills:
                        pull_one()
                    norm_outproj_pipelined(b_i, j, attn65)
                else:
                    at_bj = norm_block(attn65)
                    # age-gate: give the rf->bcast->mul chain ~4 k-tile
                    # iterations before out-proj consumes at_bj
                    for cost, u in outproj_units(b_i, j, at_bj):
                        fills.append((tick + 4, cost, u))
        while proj_q or fills:
            if not pull_one():
                tick += 1

    nc.compile()
    return nc


_CACHE = {}


def _get_program(seq=S):
    if seq not in _CACHE:
        _CACHE[seq] = build_program(seq)
    return _CACHE[seq]


def make_in_maps(x, W_qkv, b_qkv, seq=S):
    bf16 = ml_dtypes.bfloat16
    s_tot = B * seq
    xT = np.ascontiguousarray(
        x.reshape(s_tot, N_EMBED).T).astype(bf16)
    in_maps = []
    for c in range(N_CORES):
        csl = slice(DCORE * c, DCORE * (c + 1))
        in_maps.append({
            "xT": xT,
            "wq": np.ascontiguousarray(W_qkv[:, csl]).astype(bf16),
            "wk": np.ascontiguousarray(W_qkv[:, N_EMBED:][:, csl]).astype(bf16),
            "wv": np.ascontiguousarray(W_qkv[:, 2 * N_EMBED:][:, csl]).astype(bf16),
            "bq": np.ascontiguousarray(
                b_qkv[csl].reshape(DCORE, 1)).astype(np.float32),
            "bk": np.ascontiguousarray(
                b_qkv[N_EMBED:][csl].reshape(DCORE, 1)).astype(np.float32),
            "wout": None,  # filled by caller
        })
    return in_maps


def kernel(x, W_qkv, b_qkv, W_out, b_out):
    x = np.asarray(x, dtype=np.float32)
    W_qkv = np.asarray(W_qkv, dtype=np.float32)
    b_qkv = np.asarray(b_qkv, dtype=np.float32)
    W_out = np.asarray(W_out, dtype=np.float32)
    b_out = np.asarray(b_out, dtype=np.float32)

    nc = _get_program(S)
    in_maps = make_in_maps(x, W_qkv, b_qkv, S)
    bf16 = ml_dtypes.bfloat16
    for c in range(N_CORES):
        csl = slice(DCORE * c, DCORE * (c + 1))
        in_maps[c]["wout"] = np.ascontiguousarray(W_out[csl, :]).astype(bf16)

    res = run_bass_kernel_spmd(nc, in_maps, core_ids=list(range(N_CORES)))
    y = np.zeros((B * S, N_EMBED), dtype=np.float32)
    for r in res.results:
        y += r["y"].astype(np.float32)
    # bias + v-bias folded through W_out (softmax rows sum to 1)
    y += b_out[None, :] + b_qkv[2 * N_EMBED:] @ W_out
    return y.reshape(B, S, N_EMBED)
